# revision 12
# baseline (speedup 1.0000x reference)
"""Trainium2 Bass kernel for the EnsembleFeatureLoss OT problem (fp8 v4).

Math (per ensemble member e of E=4):
  s = l2norm_rows(gts[e]); t = l2norm_rows(feats[e])      # [4096, 1024]
  sim = s @ t.T                                            # [4096, 4096]
  Sinkhorn converges in exactly 2 iterations for this regime (device
  outputs are re-checked on the host, with a full numpy fallback):
    r1 = u / rowsum(K);  c1 = v / (K.T @ r1)
    r2 = u / (K @ c1);   c2 = v / (K.T @ r2)
  loss_e = sum(outer(r2, c2) * K * sim) = (1/N) * sum_n Z[n]/Y2[n]
  with Y2[n] = sum_m r2 K,  Z[n] = sum_m r2 K sim.

Everything on-device is scale-invariant (the transport plan T is
invariant under K -> lam*K and any rescaling of r/c), so the kernel
uses K' = exp(10*sim) (no -10 bias) and rescales r/c into fp8-friendly
ranges; the host combine (1/N)*sum(Z/Y2) is unchanged.

Distribution: 8 cores = 4 members x 2 row-halves (2048 rows each).
Per core, single fused schedule, all intermediates resident in SBUF:
  - host: row-normalize s/t in fp32, cast to fp8e4m3, DoubleRow layout.
  - pass A (PE-bound): fp8 DoubleRow matmuls (2x) write sim slices into
    one persistent 4-bank PSUM tile (subtile deps give a slice-level
    wavefront); ACT exp(10*sim) -> bf16 K row block with fused rowsum
    accum -> R1; DVE casts K to the fp8 K' store and writes
    KS = sim*K' fp8; Y1 accumulated on the PE (plain fp8 matmuls into
    packed PSUM regions).
  - pair AllReduce of Y1 (8KB bf16) -> C1 fp8 broadcast.
  - pass B: rowdot = sum(K'*C1) via one DVE mul + split DVE/ACT
    reduce -> R2; Y2/Z accumulated on the PE from the fp8 K'/KS stores.
Host combines per-core [4096] partial vectors and does the 4-scalar
ensemble weighting.
"""

import numpy as np
import ml_dtypes

FP8 = ml_dtypes.float8_e4m3  # TRN e4m3 (max 240)

E = 4
M = 4096
N = 4096
D = 1024
P = 128
NCORES = 8
MHALF = M // 2              # rows per core
CH = 512                    # psum bank width (f32)
NT_M = MHALF // P           # 16 row tiles per core
NPAIR = NT_M // 2           # 8 fp8-store row-tile pairs
ND2 = D // 256              # 4 contraction pair-blocks
NCH = N // CH               # 8 column chunks
HN = N // 2
SL = 1024                   # ACT/DVE slice width in pass A

# on-device scales (all cancel in the transport plan; see module doc)
ALPHA = float(M)            # R1 = ALPHA / rowsum(K')
GAMMA = float(M)            # C1 = GAMMA / Y1
DELTA = float(M)            # R2 = DELTA / rowdot
# host-side conversion of device R1/R2 back to reference r1/r2
R_SCALE = float(np.exp(10.0) / (ALPHA * M))

_CACHE = {}


def build_bass(ncores=NCORES):
    import concourse.mybir as mybir
    import concourse.tile as tile
    from concourse import bacc
    from concourse.bass import ts

    dt = mybir.dt
    f32, bf16, f8 = dt.float32, dt.bfloat16, dt.float8e4
    Alu = mybir.AluOpType
    Act = mybir.ActivationFunctionType
    PM = mybir.MatmulPerfMode.DoubleRow
    AxX = mybir.AxisListType.X

    rg = [[i, i + 1] for i in range(0, ncores, 2)]

    nc = bacc.Bacc("TRN2", target_bir_lowering=False, debug=False,
                   num_devices=ncores)
    sT8 = nc.declare_dram_parameter("sT8", [ND2 * P, 2 * MHALF], f8,
                                    isOutput=False)
    tT8 = nc.declare_dram_parameter("tT8", [ND2 * P, 2 * N], f8,
                                    isOutput=False)
    vecs = nc.declare_dram_parameter("vecs", [2, N], bf16, isOutput=True)
    r1o = nc.declare_dram_parameter("r1o", [P, NT_M], f32, isOutput=True)
    r2o = nc.declare_dram_parameter("r2o", [P, NT_M], f32, isOutput=True)

    with tile.TileContext(nc) as tc:
        with (
            tc.tile_pool(name="persist", bufs=1) as pp,
            tc.tile_pool(name="opt", bufs=4) as optp,      # tT fp8 blocks
            tc.tile_pool(name="scr", bufs=4) as scrp,      # sT fp8 blocks
            tc.tile_pool(name="kst", bufs=8) as kstp,      # K' fp8 store
            tc.tile_pool(name="kss", bufs=8) as kssp,      # K'*sim fp8 store
            tc.tile_pool(name="vec", bufs=1) as vecp,      # [1,N] staging
            tc.tile_pool(name="kbf", bufs=2) as kbfp,      # [P,N] bf16 rows
            tc.tile_pool(name="rq", bufs=2) as rqp,        # r fp8 scalars
            tc.tile_pool(name="sm", bufs=8) as smp,        # tiny stats
            tc.tile_pool(name="pm", bufs=1, space="PSUM") as ppm,
            tc.tile_pool(name="py", bufs=1, space="PSUM") as psy,
            tc.tile_pool(name="dram", bufs=1, space="DRAM") as dp,
        ):
            # ---- dram scratch for the pair AllReduce ----
            y1_in = dp.tile([1, N], bf16, name="y1_in", tag="y1_in")
            y1_out = dp.tile([1, N], bf16, name="y1_out", tag="y1_out")
            c1_d = dp.tile([1, N], f8, name="c1_d", tag="c1_d")

            # ---- persistent sbuf ----
            tTq = [optp.tile([P, 2, N], f8, name=f"tTq{b}", tag="opt")
                   for b in range(ND2)]
            sTq = [scrp.tile([P, 2, MHALF], f8, name=f"sTq{b}", tag="scr")
                   for b in range(ND2)]
            Kst = [kstp.tile([P, 2, N], f8, name=f"Kst{q}", tag="kst")
                   for q in range(NPAIR)]
            KSst = [kssp.tile([P, 2, N], f8, name=f"KSst{q}", tag="kss")
                    for q in range(NPAIR)]
            c1_bc = pp.tile([P, N], f8, name="c1_bc", tag="c1_bc")
            r1buf = pp.tile([P, NT_M], f32, name="r1buf", tag="r1buf")
            r2buf = pp.tile([P, NT_M], f32, name="r2buf", tag="r2buf")

            # ---- persistent psum: one 4-bank sim tile + one 4-bank
            #      accumulator tile (8 packed [1,512] regions each) ----
            pm = ppm.tile([P, 4 * CH], f32, name="pm", tag="pm")
            pY1 = psy.tile([P, 4 * CH], f32, name="pY1", tag="py")

            with nc.named_scope("load"):
                for b in range(ND2):
                    nc.sync.dma_start(
                        tTq[b][:], tT8[ts(b, P), :].rearrange(
                            "p (j m) -> p j m", j=2))
                    nc.scalar.dma_start(
                        sTq[b][:], sT8[ts(b, P), :].rearrange(
                            "p (j m) -> p j m", j=2))
                nc.vector.memset(pY1[:], 0.0)

            r1q8s = [None] * NT_M

            def acc_region(ptile, ni):
                b, pt = ni // 2, (ni % 2) * 32
                return ptile[pt:pt + 1, ts(b, CH)], (0, pt)

            def emit_y1(mi):
                q, j = divmod(mi, 2)
                for ni in range(NCH):
                    out, tpos = acc_region(pY1, ni)
                    nc.tensor.matmul(
                        out, r1q8s[mi][:], Kst[q][:, j:j + 1, ts(ni, CH)],
                        start=False, stop=(mi == NT_M - 1),
                        skip_group_check=True, tile_position=tpos)

            # ---- pass A ----
            with nc.named_scope("passA"):
                for mi in range(NT_M):
                    q, j = divmod(mi, 2)
                    rs4 = smp.tile([P, 4], f32, name="rs4", tag="sm8")
                    Kbf = kbfp.tile([P, N], bf16, name="Kbf", tag="kbf")
                    for h in range(2):          # two 4-bank fills per mi
                        for c in range(4):      # 512-col chunks
                            ni = 4 * h + c
                            for dd in range(ND2):
                                nc.tensor.matmul(
                                    pm[:, ts(c, CH)],
                                    sTq[dd][:, :, ts(mi, P)],
                                    tTq[dd][:, :, ts(ni, CH)],
                                    start=(dd == 0), stop=(dd == ND2 - 1),
                                    perf_mode=PM, skip_group_check=True)
                        for g in range(2):      # 1024-wide ACT/DVE slices
                            i = 2 * h + g
                            psl = pm[:, ts(g, SL)]
                            ksl = Kbf[:, ts(i, SL)]
                            nc.scalar.activation(
                                ksl, psl, Act.Exp, scale=10.0,
                                accum_out=rs4[:, i:i + 1])
                            nc.vector.tensor_mul(
                                KSst[q][:, j:j + 1, ts(i, SL)], psl, ksl)
                    nc.vector.tensor_copy(Kst[q][:, j:j + 1, :], Kbf[:])
                    rowsum = smp.tile([P, 1], f32, name="rowsum", tag="sm")
                    nc.vector.tensor_reduce(rowsum[:], rs4[:], AxX, Alu.add)
                    rinv = smp.tile([P, 1], f32, name="rinv", tag="sm")
                    nc.vector.reciprocal(rinv[:], rowsum[:])
                    nc.vector.tensor_scalar_mul(r1buf[:, mi:mi + 1], rinv[:],
                                                ALPHA)
                    r1q8s[mi] = rqp.tile([P, 1], f8, name="r1q", tag="r1q")
                    nc.vector.tensor_copy(r1q8s[mi][:], r1buf[:, mi:mi + 1])
                    # Y1 matmuls one row-tile behind: PE never waits on r1
                    if mi >= 1:
                        emit_y1(mi - 1)
                emit_y1(NT_M - 1)

            # ---- Y1 pair AllReduce -> C1 broadcast ----
            with nc.named_scope("ar"):
                y1sb = vecp.tile([1, N], bf16, name="y1sb", tag="vec")
                for ni in range(NCH):
                    src, _ = acc_region(pY1, ni)
                    nc.scalar.copy(y1sb[0:1, ts(ni, CH)], src)
                nc.gpsimd.dma_start(y1_in[:], y1sb[0:1, :])
                nc.gpsimd.collective_compute(
                    "AllReduce", Alu.add, replica_groups=rg,
                    ins=[y1_in.opt()], outs=[y1_out.opt()])
                nq = N // P
                y1r = smp.tile([P, nq], bf16, name="y1r", tag="sm32")
                nc.gpsimd.dma_start(
                    y1r[:], y1_out[0:1, :].rearrange("a (q p) -> (a p) q",
                                                     p=P))
                y1ri = smp.tile([P, nq], f32, name="y1ri", tag="sm32f")
                nc.vector.reciprocal(y1ri[:], y1r[:])
                c1r = smp.tile([P, nq], f8, name="c1r", tag="sm32h")
                nc.vector.tensor_scalar_mul(c1r[:], y1ri[:], GAMMA)
                nc.gpsimd.dma_start(
                    c1_d[0:1, :].rearrange("a (q p) -> (a p) q", p=P), c1r[:])
                nc.scalar.dma_start(c1_bc[:],
                                    c1_d[0:1, :].to_broadcast((P, N)))

            # ---- pass B ----
            with nc.named_scope("passB"):
                pY2 = ppm.tile([P, 4 * CH], f32, name="pY2", tag="pm")
                pZ = psy.tile([P, 4 * CH], f32, name="pZ", tag="py")
                nc.vector.memset(pY2[:], 0.0)
                nc.vector.memset(pZ[:], 0.0)

                r2q8s = [None] * NT_M

                def emit_y2z(mi):
                    q, j = divmod(mi, 2)
                    for ni in range(NCH):
                        out, tpos = acc_region(pY2, ni)
                        nc.tensor.matmul(
                            out, r2q8s[mi][:], Kst[q][:, j:j + 1, ts(ni, CH)],
                            start=False, stop=(mi == NT_M - 1),
                            skip_group_check=True, tile_position=tpos)
                        out, tpos = acc_region(pZ, ni)
                        nc.tensor.matmul(
                            out, r2q8s[mi][:],
                            KSst[q][:, j:j + 1, ts(ni, CH)],
                            start=False, stop=(mi == NT_M - 1),
                            skip_group_check=True, tile_position=tpos)

                for mi in range(NT_M):
                    q, j = divmod(mi, 2)
                    kc = kbfp.tile([P, N], bf16, name="kc", tag="kbf")
                    nc.vector.tensor_mul(kc[:], Kst[q][:, j:j + 1, :],
                                         c1_bc[:])
                    rda = smp.tile([P, 1], f32, name="rda", tag="sm")
                    rdb = smp.tile([P, 1], f32, name="rdb", tag="sm")
                    rowdot = smp.tile([P, 1], f32, name="rowdot", tag="sm")
                    nc.vector.tensor_reduce(rda[:], kc[:, 0:HN], AxX, Alu.add)
                    nc.scalar.activation(kc[:, HN:N], kc[:, HN:N], Act.Copy,
                                         accum_out=rdb[:])
                    nc.vector.tensor_add(rowdot[:], rda[:], rdb[:])
                    rdinv = smp.tile([P, 1], f32, name="rdinv", tag="sm")
                    nc.vector.reciprocal(rdinv[:], rowdot[:])
                    nc.vector.tensor_scalar_mul(r2buf[:, mi:mi + 1], rdinv[:],
                                                DELTA)
                    r2q8s[mi] = rqp.tile([P, 1], f8, name="r2q", tag="r2q")
                    nc.vector.tensor_copy(r2q8s[mi][:], r2buf[:, mi:mi + 1])
                    if mi >= 1:
                        emit_y2z(mi - 1)
                emit_y2z(NT_M - 1)

            # ---- outputs ----
            with nc.named_scope("epi"):
                ysb = vecp.tile([1, N], bf16, name="ysb", tag="vec")
                for ni in range(NCH):
                    src, _ = acc_region(pY2, ni)
                    nc.scalar.copy(ysb[0:1, ts(ni, CH)], src)
                nc.sync.dma_start(vecs[0:1, :], ysb[0:1, :])
                zsb = vecp.tile([1, N], bf16, name="zsb", tag="vec")
                for ni in range(NCH):
                    src, _ = acc_region(pZ, ni)
                    nc.scalar.copy(zsb[0:1, ts(ni, CH)], src)
                nc.sync.dma_start(vecs[1:2, :], zsb[0:1, :])
                nc.gpsimd.dma_start(r1o[:, :], r1buf[:])
                nc.gpsimd.dma_start(r2o[:, :], r2buf[:])

    return nc


def _dr_layout(xT):
    """[D, F] -> DoubleRow DRAM layout [ND2*128, 2*F]."""
    d, f = xT.shape
    return np.ascontiguousarray(
        xT.reshape(ND2, 2, P, f).transpose(0, 2, 1, 3).reshape(ND2 * P, 2 * f))


def _make_in_maps(gts, feats):
    in_maps = []
    t8 = []
    for e in range(E):
        t = feats[e]
        tn = t / np.maximum(np.linalg.norm(t, axis=1, keepdims=True), 1e-12)
        t8.append(_dr_layout(np.ascontiguousarray(tn.T).astype(FP8)))
    for core in range(NCORES):
        e, h = divmod(core, 2)
        s = gts[e][h * MHALF:(h + 1) * MHALF]
        sn = s / np.maximum(np.linalg.norm(s, axis=1, keepdims=True), 1e-12)
        in_maps.append({
            "sT8": _dr_layout(np.ascontiguousarray(sn.T).astype(FP8)),
            "tT8": t8[e],
        })
    return in_maps


def _ensemble(losses, prev_losses):
    l = np.asarray(losses, np.float64)
    ratio = l / (np.asarray(prev_losses, np.float64) + 1e-8)
    w = np.exp(ratio / 1.0)
    w = w / np.sum(w) * l.shape[0]
    return np.float32(np.sum(w * l))


def _numpy_reference(gts, feats, prev_losses):
    """Faithful float32 fallback, used only if the on-device convergence
    check is violated (never observed for this problem's regime)."""
    losses = []
    for e in range(gts.shape[0]):
        s = gts[e] / np.maximum(
            np.linalg.norm(gts[e], axis=1, keepdims=True), 1e-12)
        t = feats[e] / np.maximum(
            np.linalg.norm(feats[e], axis=1, keepdims=True), 1e-12)
        sim = (s @ t.T).astype(np.float32)
        K = np.exp(-(1.0 - sim) / 0.1)
        m, n = sim.shape
        u = np.full(m, 1.0 / m, np.float32)
        v = np.full(n, 1.0 / n, np.float32)
        r = np.ones(m, np.float32)
        c = np.ones(n, np.float32)
        err = np.inf
        for _ in range(100):
            if err < 0.01:
                break
            r_new = u / (K @ c)
            c = v / (K.T @ r_new)
            err = float(np.mean(np.abs(r_new - r)))
            r = r_new
        losses.append(np.sum(np.outer(r, c) * K * sim))
    return _ensemble(losses, prev_losses)


def _run(gts, feats, trace=False):
    from concourse.bass_utils import run_bass_kernel_spmd
    if "nc" not in _CACHE:
        nc = build_bass()
        nc.finalize()
        _CACHE["nc"] = nc
    in_maps = _make_in_maps(gts, feats)
    return run_bass_kernel_spmd(_CACHE["nc"], in_maps,
                                list(range(NCORES)), trace=trace)


def _combine(results, gts, feats, prev_losses):
    losses = []
    ok = True
    for e in range(E):
        a, b = results[2 * e], results[2 * e + 1]
        Y2 = a["vecs"][0].astype(np.float64) + b["vecs"][0].astype(np.float64)
        Z = a["vecs"][1].astype(np.float64) + b["vecs"][1].astype(np.float64)
        c2 = (1.0 / N) / Y2
        losses.append(np.sum(c2 * Z))
        r1 = np.concatenate([a["r1o"].T.reshape(-1),
                             b["r1o"].T.reshape(-1)]) * R_SCALE
        r2 = np.concatenate([a["r2o"].T.reshape(-1),
                             b["r2o"].T.reshape(-1)]) * R_SCALE
        err1 = np.mean(np.abs(r1 - 1.0))
        err2 = np.mean(np.abs(r2 - r1))
        if not (err1 >= 0.01 and err2 < 0.01):
            ok = False
    if not ok:
        return _numpy_reference(gts, feats, prev_losses)
    return _ensemble(losses, prev_losses)


def kernel(gts, feats, prev_losses):
    gts = np.asarray(gts, np.float32)
    feats = np.asarray(feats, np.float32)
    prev_losses = np.asarray(prev_losses, np.float32)
    res = _run(gts, feats)
    return _combine(res.results, gts, feats, prev_losses)


# revision 13
# speedup vs baseline: 1.6331x; 1.6331x over previous
"""Trainium2 Bass kernel for the EnsembleFeatureLoss OT problem (fp8 v4).

Math (per ensemble member e of E=4):
  s = l2norm_rows(gts[e]); t = l2norm_rows(feats[e])      # [4096, 1024]
  sim = s @ t.T                                            # [4096, 4096]
  Sinkhorn converges in exactly 2 iterations for this regime (device
  outputs are re-checked on the host, with a full numpy fallback):
    r1 = u / rowsum(K);  c1 = v / (K.T @ r1)
    r2 = u / (K @ c1);   c2 = v / (K.T @ r2)
  loss_e = sum(outer(r2, c2) * K * sim) = (1/N) * sum_n Z[n]/Y2[n]
  with Y2[n] = sum_m r2 K,  Z[n] = sum_m r2 K sim.

Everything on-device is scale-invariant (the transport plan T is
invariant under K -> lam*K and any rescaling of r/c), so the kernel
uses K' = exp(10*sim) (no -10 bias) and rescales r/c into fp8-friendly
ranges; the host combine (1/N)*sum(Z/Y2) is unchanged.

Distribution: 8 cores = 4 members x 2 row-halves (2048 rows each).
Per core, single fused schedule, all intermediates resident in SBUF:
  - host: row-normalize s/t in fp32, cast to fp8e4m3, DoubleRow layout.
  - pass A (PE-bound): fp8 DoubleRow matmuls (2x) write sim slices into
    one persistent 4-bank PSUM tile (subtile deps give a slice-level
    wavefront); ACT exp(10*sim) -> bf16 K row block with fused rowsum
    accum -> R1; DVE casts K to the fp8 K' store and writes
    KS = sim*K' fp8; Y1 accumulated on the PE (plain fp8 matmuls into
    packed PSUM regions).
  - pair AllReduce of Y1 (8KB bf16) -> C1 fp8 broadcast.
  - pass B: rowdot = sum(K'*C1) via one DVE mul + split DVE/ACT
    reduce -> R2; Y2/Z accumulated on the PE from the fp8 K'/KS stores.
Host combines per-core [4096] partial vectors and does the 4-scalar
ensemble weighting.
"""

import numpy as np
import ml_dtypes

FP8 = ml_dtypes.float8_e4m3  # TRN e4m3 (max 240)

E = 4
M = 4096
N = 4096
D = 1024
P = 128
NCORES = 8
MHALF = M // 2              # rows per core
CH = 512                    # psum bank width (f32)
NT_M = MHALF // P           # 16 row tiles per core
NPAIR = NT_M // 2           # 8 fp8-store row-tile pairs
ND2 = D // 256              # 4 contraction pair-blocks
NCH = N // CH               # 8 column chunks
HN = N // 2
SL = 1024                   # ACT/DVE slice width in pass A

# on-device scales (all cancel in the transport plan; see module doc)
ALPHA = float(M)            # R1 = ALPHA / rowsum(K')
GAMMA = float(M)            # C1 = GAMMA / Y1
DELTA = float(M)            # R2 = DELTA / rowdot
# host-side conversion of device R1/R2 back to reference r1/r2
R_SCALE = float(np.exp(10.0) / (ALPHA * M))

_CACHE = {}


def build_bass(ncores=NCORES):
    import concourse.mybir as mybir
    import concourse.tile as tile
    from concourse import bacc
    from concourse.bass import ts

    dt = mybir.dt
    f32, bf16, f8 = dt.float32, dt.bfloat16, dt.float8e4
    Alu = mybir.AluOpType
    Act = mybir.ActivationFunctionType
    PM = mybir.MatmulPerfMode.DoubleRow
    AxX = mybir.AxisListType.X

    rg = [[i, i + 1] for i in range(0, ncores, 2)]

    nc = bacc.Bacc("TRN2", target_bir_lowering=False, debug=False,
                   num_devices=ncores)
    sT8 = nc.declare_dram_parameter("sT8", [ND2 * P, 2 * MHALF], f8,
                                    isOutput=False)
    tT8 = nc.declare_dram_parameter("tT8", [ND2 * P, 2 * N], f8,
                                    isOutput=False)
    vecs = nc.declare_dram_parameter("vecs", [2, N], bf16, isOutput=True)
    r1o = nc.declare_dram_parameter("r1o", [P, NT_M], f32, isOutput=True)
    r2o = nc.declare_dram_parameter("r2o", [P, NT_M], f32, isOutput=True)

    with tile.TileContext(nc) as tc:
        with (
            tc.tile_pool(name="persist", bufs=1) as pp,
            tc.tile_pool(name="opt", bufs=4) as optp,      # tT fp8 blocks
            tc.tile_pool(name="scr", bufs=4) as scrp,      # sT fp8 blocks
            tc.tile_pool(name="kst", bufs=8) as kstp,      # K' fp8 store
            tc.tile_pool(name="kss", bufs=8) as kssp,      # K'*sim fp8 store
            tc.tile_pool(name="vec", bufs=1) as vecp,      # [1,N] staging
            tc.tile_pool(name="kbf", bufs=2) as kbfp,      # [P,N] bf16 rows
            tc.tile_pool(name="rq", bufs=2) as rqp,        # r fp8 scalars
            tc.tile_pool(name="sm", bufs=8) as smp,        # tiny stats
            tc.tile_pool(name="pm", bufs=2, space="PSUM") as ppm,
            tc.tile_pool(name="py", bufs=1, space="PSUM") as psy,
            tc.tile_pool(name="dram", bufs=1, space="DRAM") as dp,
        ):
            # ---- dram scratch for the pair AllReduce ----
            y1_in = dp.tile([1, N], bf16, name="y1_in", tag="y1_in")
            y1_out = dp.tile([1, N], bf16, name="y1_out", tag="y1_out")
            c1_d = dp.tile([1, N], f8, name="c1_d", tag="c1_d")

            # ---- persistent sbuf ----
            tTq = [optp.tile([P, 2, N], f8, name=f"tTq{b}", tag="opt")
                   for b in range(ND2)]
            sTq = [scrp.tile([P, 2, MHALF], f8, name=f"sTq{b}", tag="scr")
                   for b in range(ND2)]
            Kst = [kstp.tile([P, 2, N], f8, name=f"Kst{q}", tag="kst")
                   for q in range(NPAIR)]
            KSst = [kssp.tile([P, 2, N], f8, name=f"KSst{q}", tag="kss")
                    for q in range(NPAIR)]
            c1_bc = pp.tile([P, N], f8, name="c1_bc", tag="c1_bc")
            r1buf = pp.tile([P, NT_M], f32, name="r1buf", tag="r1buf")
            r2buf = pp.tile([P, NT_M], f32, name="r2buf", tag="r2buf")

            # ---- psum: rotating 2-bank sim tiles + one 4-bank
            #      accumulator tile (8 packed [1,512] regions) ----
            pY1 = psy.tile([P, 4 * CH], f32, name="pY1", tag="py")

            with nc.named_scope("load"):
                for b in range(ND2):
                    nc.sync.dma_start(
                        tTq[b][:], tT8[ts(b, P), :].rearrange(
                            "p (j m) -> p j m", j=2))
                    nc.scalar.dma_start(
                        sTq[b][:], sT8[ts(b, P), :].rearrange(
                            "p (j m) -> p j m", j=2))
                nc.vector.memset(pY1[:], 0.0)

            r1q8s = [None] * NT_M

            def acc_region(ptile, ni):
                b, pt = ni // 2, (ni % 2) * 32
                return ptile[pt:pt + 1, ts(b, CH)], (0, pt)

            def emit_y1(mi):
                q, j = divmod(mi, 2)
                for ni in range(NCH):
                    out, tpos = acc_region(pY1, ni)
                    nc.tensor.matmul(
                        out, r1q8s[mi][:], Kst[q][:, j:j + 1, ts(ni, CH)],
                        start=False, stop=(mi == NT_M - 1),
                        skip_group_check=True, tile_position=tpos)

            # ---- pass A ----
            with nc.named_scope("passA"):
                for mi in range(NT_M):
                    q, j = divmod(mi, 2)
                    rs4 = smp.tile([P, 4], f32, name="rs4", tag="sm8")
                    Kbf = kbfp.tile([P, N], bf16, name="Kbf", tag="kbf")
                    for i in range(4):          # 1024-wide slices per mi
                        pm = ppm.tile([P, SL], f32, name="pm", tag="pm")
                        for c in range(2):      # 512-col chunks
                            ni = 2 * i + c
                            for dd in range(ND2):
                                nc.tensor.matmul(
                                    pm[:, ts(c, CH)],
                                    sTq[dd][:, :, ts(mi, P)],
                                    tTq[dd][:, :, ts(ni, CH)],
                                    start=(dd == 0), stop=(dd == ND2 - 1),
                                    perf_mode=PM, skip_group_check=True)
                        ksl = Kbf[:, ts(i, SL)]
                        nc.scalar.activation(ksl, pm[:], Act.Exp, scale=10.0,
                                             accum_out=rs4[:, i:i + 1])
                        nc.vector.tensor_mul(
                            KSst[q][:, j:j + 1, ts(i, SL)], pm[:], ksl)
                    nc.vector.tensor_copy(Kst[q][:, j:j + 1, :], Kbf[:])
                    rowsum = smp.tile([P, 1], f32, name="rowsum", tag="sm")
                    nc.vector.tensor_reduce(rowsum[:], rs4[:], AxX, Alu.add)
                    rinv = smp.tile([P, 1], f32, name="rinv", tag="sm")
                    nc.vector.reciprocal(rinv[:], rowsum[:])
                    nc.vector.tensor_scalar_mul(r1buf[:, mi:mi + 1], rinv[:],
                                                ALPHA)
                    r1q8s[mi] = rqp.tile([P, 1], f8, name="r1q", tag="r1q")
                    nc.vector.tensor_copy(r1q8s[mi][:], r1buf[:, mi:mi + 1])
                    # Y1 matmuls one row-tile behind: PE never waits on r1
                    if mi >= 1:
                        emit_y1(mi - 1)
                emit_y1(NT_M - 1)

            # ---- Y1 pair AllReduce -> C1 broadcast ----
            with nc.named_scope("ar"):
                y1sb = vecp.tile([1, N], bf16, name="y1sb", tag="vec")
                for ni in range(NCH):
                    src, _ = acc_region(pY1, ni)
                    nc.scalar.copy(y1sb[0:1, ts(ni, CH)], src)
                nc.gpsimd.dma_start(y1_in[:], y1sb[0:1, :])
                nc.gpsimd.collective_compute(
                    "AllReduce", Alu.add, replica_groups=rg,
                    ins=[y1_in.opt()], outs=[y1_out.opt()])
                nq = N // P
                y1r = smp.tile([P, nq], bf16, name="y1r", tag="sm32")
                nc.gpsimd.dma_start(
                    y1r[:], y1_out[0:1, :].rearrange("a (q p) -> (a p) q",
                                                     p=P))
                y1ri = smp.tile([P, nq], f32, name="y1ri", tag="sm32f")
                nc.vector.reciprocal(y1ri[:], y1r[:])
                c1r = smp.tile([P, nq], f8, name="c1r", tag="sm32h")
                nc.vector.tensor_scalar_mul(c1r[:], y1ri[:], GAMMA)
                nc.gpsimd.dma_start(
                    c1_d[0:1, :].rearrange("a (q p) -> (a p) q", p=P), c1r[:])
                nc.scalar.dma_start(c1_bc[:],
                                    c1_d[0:1, :].to_broadcast((P, N)))

            # ---- pass B ----
            with nc.named_scope("passB"):
                pY2h = [ppm.tile([P, SL], f32, name=f"pY2{h}", tag="pm")
                        for h in range(2)]
                pZ = psy.tile([P, 4 * CH], f32, name="pZ", tag="py")
                for t in pY2h:
                    nc.vector.memset(t[:], 0.0)
                nc.vector.memset(pZ[:], 0.0)

                def y2_region(ni):
                    h, r = divmod(ni, 4)
                    b, pt = r // 2, (r % 2) * 32
                    return pY2h[h][pt:pt + 1, ts(b, CH)], (0, pt)

                r2q8s = [None] * NT_M

                def emit_y2z(mi):
                    q, j = divmod(mi, 2)
                    for ni in range(NCH):
                        out, tpos = y2_region(ni)
                        nc.tensor.matmul(
                            out, r2q8s[mi][:], Kst[q][:, j:j + 1, ts(ni, CH)],
                            start=False, stop=(mi == NT_M - 1),
                            skip_group_check=True, tile_position=tpos)
                        out, tpos = acc_region(pZ, ni)
                        nc.tensor.matmul(
                            out, r2q8s[mi][:],
                            KSst[q][:, j:j + 1, ts(ni, CH)],
                            start=False, stop=(mi == NT_M - 1),
                            skip_group_check=True, tile_position=tpos)

                for mi in range(NT_M):
                    q, j = divmod(mi, 2)
                    kc = kbfp.tile([P, N], bf16, name="kc", tag="kbf")
                    nc.vector.tensor_mul(kc[:], Kst[q][:, j:j + 1, :],
                                         c1_bc[:])
                    rda = smp.tile([P, 1], f32, name="rda", tag="sm")
                    rdb = smp.tile([P, 1], f32, name="rdb", tag="sm")
                    rowdot = smp.tile([P, 1], f32, name="rowdot", tag="sm")
                    nc.vector.tensor_reduce(rda[:], kc[:, 0:HN], AxX, Alu.add)
                    nc.scalar.activation(kc[:, HN:N], kc[:, HN:N], Act.Copy,
                                         accum_out=rdb[:])
                    nc.vector.tensor_add(rowdot[:], rda[:], rdb[:])
                    rdinv = smp.tile([P, 1], f32, name="rdinv", tag="sm")
                    nc.vector.reciprocal(rdinv[:], rowdot[:])
                    nc.vector.tensor_scalar_mul(r2buf[:, mi:mi + 1], rdinv[:],
                                                DELTA)
                    r2q8s[mi] = rqp.tile([P, 1], f8, name="r2q", tag="r2q")
                    nc.vector.tensor_copy(r2q8s[mi][:], r2buf[:, mi:mi + 1])
                    if mi >= 1:
                        emit_y2z(mi - 1)
                emit_y2z(NT_M - 1)

            # ---- outputs ----
            with nc.named_scope("epi"):
                ysb = vecp.tile([1, N], bf16, name="ysb", tag="vec")
                for ni in range(NCH):
                    src, _ = y2_region(ni)
                    nc.scalar.copy(ysb[0:1, ts(ni, CH)], src)
                nc.sync.dma_start(vecs[0:1, :], ysb[0:1, :])
                zsb = vecp.tile([1, N], bf16, name="zsb", tag="vec")
                for ni in range(NCH):
                    src, _ = acc_region(pZ, ni)
                    nc.scalar.copy(zsb[0:1, ts(ni, CH)], src)
                nc.sync.dma_start(vecs[1:2, :], zsb[0:1, :])
                nc.gpsimd.dma_start(r1o[:, :], r1buf[:])
                nc.gpsimd.dma_start(r2o[:, :], r2buf[:])

    return nc


def _dr_layout(xT):
    """[D, F] -> DoubleRow DRAM layout [ND2*128, 2*F]."""
    d, f = xT.shape
    return np.ascontiguousarray(
        xT.reshape(ND2, 2, P, f).transpose(0, 2, 1, 3).reshape(ND2 * P, 2 * f))


def _make_in_maps(gts, feats):
    in_maps = []
    t8 = []
    for e in range(E):
        t = feats[e]
        tn = t / np.maximum(np.linalg.norm(t, axis=1, keepdims=True), 1e-12)
        t8.append(_dr_layout(np.ascontiguousarray(tn.T).astype(FP8)))
    for core in range(NCORES):
        e, h = divmod(core, 2)
        s = gts[e][h * MHALF:(h + 1) * MHALF]
        sn = s / np.maximum(np.linalg.norm(s, axis=1, keepdims=True), 1e-12)
        in_maps.append({
            "sT8": _dr_layout(np.ascontiguousarray(sn.T).astype(FP8)),
            "tT8": t8[e],
        })
    return in_maps


def _ensemble(losses, prev_losses):
    l = np.asarray(losses, np.float64)
    ratio = l / (np.asarray(prev_losses, np.float64) + 1e-8)
    w = np.exp(ratio / 1.0)
    w = w / np.sum(w) * l.shape[0]
    return np.float32(np.sum(w * l))


def _numpy_reference(gts, feats, prev_losses):
    """Faithful float32 fallback, used only if the on-device convergence
    check is violated (never observed for this problem's regime)."""
    losses = []
    for e in range(gts.shape[0]):
        s = gts[e] / np.maximum(
            np.linalg.norm(gts[e], axis=1, keepdims=True), 1e-12)
        t = feats[e] / np.maximum(
            np.linalg.norm(feats[e], axis=1, keepdims=True), 1e-12)
        sim = (s @ t.T).astype(np.float32)
        K = np.exp(-(1.0 - sim) / 0.1)
        m, n = sim.shape
        u = np.full(m, 1.0 / m, np.float32)
        v = np.full(n, 1.0 / n, np.float32)
        r = np.ones(m, np.float32)
        c = np.ones(n, np.float32)
        err = np.inf
        for _ in range(100):
            if err < 0.01:
                break
            r_new = u / (K @ c)
            c = v / (K.T @ r_new)
            err = float(np.mean(np.abs(r_new - r)))
            r = r_new
        losses.append(np.sum(np.outer(r, c) * K * sim))
    return _ensemble(losses, prev_losses)


def _run(gts, feats, trace=False):
    from concourse.bass_utils import run_bass_kernel_spmd
    if "nc" not in _CACHE:
        nc = build_bass()
        nc.finalize()
        _CACHE["nc"] = nc
    in_maps = _make_in_maps(gts, feats)
    return run_bass_kernel_spmd(_CACHE["nc"], in_maps,
                                list(range(NCORES)), trace=trace)


def _combine(results, gts, feats, prev_losses):
    losses = []
    ok = True
    for e in range(E):
        a, b = results[2 * e], results[2 * e + 1]
        Y2 = a["vecs"][0].astype(np.float64) + b["vecs"][0].astype(np.float64)
        Z = a["vecs"][1].astype(np.float64) + b["vecs"][1].astype(np.float64)
        c2 = (1.0 / N) / Y2
        losses.append(np.sum(c2 * Z))
        r1 = np.concatenate([a["r1o"].T.reshape(-1),
                             b["r1o"].T.reshape(-1)]) * R_SCALE
        r2 = np.concatenate([a["r2o"].T.reshape(-1),
                             b["r2o"].T.reshape(-1)]) * R_SCALE
        err1 = np.mean(np.abs(r1 - 1.0))
        err2 = np.mean(np.abs(r2 - r1))
        if not (err1 >= 0.01 and err2 < 0.01):
            ok = False
    if not ok:
        return _numpy_reference(gts, feats, prev_losses)
    return _ensemble(losses, prev_losses)


def kernel(gts, feats, prev_losses):
    gts = np.asarray(gts, np.float32)
    feats = np.asarray(feats, np.float32)
    prev_losses = np.asarray(prev_losses, np.float32)
    res = _run(gts, feats)
    return _combine(res.results, gts, feats, prev_losses)
